# revision 32
# baseline (speedup 1.0000x reference)
"""CrissCrossAttention Trainium2 kernel (fused bf16 per-bank pipeline).

Math notes (verified in float64): the reference's column-attention einsum
('bnjid,bnkid->bnjik' applied to grid-swapped q/k/v) is an alpha-renaming
that exactly undoes the swap, so reference == 2 * row_attention:
    out = (2 * row_attn(x)) @ Wo + bo
Row attention per (batch, head, grid-row i):
    S = Q_i K_i^T * d^-0.5 ; P = softmax_k(S) ; O_i = P V_i
with grid 64x64 (n = 4096 = i*64 + j), heads=8, d=64.

Distribution: data-parallel over batch; core b handles x[b].

Design (single fused pipeline, one pass over the 8 n-banks of 512 rows;
everything is bank-local because the attention is block-diagonal in n):
  - all matmuls in bf16 (x converted fp32->bf16 on DVE/ACT a bank ahead;
    banks 0/1 instead transpose straight from fp32 into the pj PSUM tag,
    cutting the convert stage from the startup critical path; weights
    converted once at startup, scheduled off the critical path)
  - q drained from PSUM into a zero-padded block-diagonal layout qbd so
    each S matmul contracts over the full 128 partitions (2 heads per
    instruction): 256 full-width S matmuls instead of 512 half-width
  - x-transposes and P-transposes share one bf16 PSUM tag; PSUM budget
    (bank-granular, 8 x 2KB): tp16 x2 + pj x2 + sb x2 + ob x2
  - per-bank stages with software skew, interleaved sub-bank so the PE
    never waits on the ACT/DVE/GpSimd softmax chain:
      dma(bk+2) | transpose(bk+1) | project(bk) | attend(bk-1) | out(bk-2)
  - x DMAed per 128-row tile (sequential HBM reads); weight DMAs ordered
    wq before wk/wv so the first projection is never weight-gated
"""

import sys

if "/opt/trn_rl_repo" not in sys.path:
    sys.path.insert(0, "/opt/trn_rl_repo")

import numpy as np

import concourse.bass as bass
import concourse.mybir as mybir
import concourse.tile as tile
from concourse import bacc
from concourse.masks import make_identity

F32 = mybir.dt.float32
BF16 = mybir.dt.bfloat16

N = 4096
D = 512
G = 64          # grid side
NC = 4          # dim chunks of 128 (2 heads each)
NB = 8          # n banks of 512
SCALE = G ** -0.5


def build_kernel(n_cores: int = 8):
    nc = bacc.Bacc("TRN2", target_bir_lowering=False, debug=False,
                   num_devices=n_cores)

    x_d = nc.dram_tensor("x", [N, D], F32, kind="ExternalInput").ap()
    wq_d = nc.dram_tensor("Wq", [D, D], F32, kind="ExternalInput").ap()
    wk_d = nc.dram_tensor("Wk", [D, D], F32, kind="ExternalInput").ap()
    wv_d = nc.dram_tensor("Wv", [D, D], F32, kind="ExternalInput").ap()
    wo_d = nc.dram_tensor("Wo", [D, D], F32, kind="ExternalInput").ap()
    bo_d = nc.dram_tensor("bo", [D], F32, kind="ExternalInput").ap()
    out_d = nc.dram_tensor("out", [N, D], F32, kind="ExternalOutput").ap()

    with tile.TileContext(nc) as tc:
        with (
            tc.tile_pool(name="consts", bufs=1) as consts,
            tc.tile_pool(name="bank", bufs=1) as bankp,
            tc.tile_pool(name="xin", bufs=12) as xin,
            tc.tile_pool(name="xbf", bufs=12) as xbfp,
            tc.tile_pool(name="attn", bufs=1) as attn,
            tc.tile_pool(name="outsb", bufs=8) as outp,
            tc.tile_pool(name="psum", bufs=1, space="PSUM") as psum,
        ):
            ident_bf = consts.tile([128, 128], BF16, tag="idb")
            make_identity(nc, ident_bf)
            ident_f32 = consts.tile([128, 128], F32, tag="idf")
            make_identity(nc, ident_f32)

            # ---- weights: staging tiles; DMA + convert in the schedule
            w_dram = {"wq": wq_d, "wk": wk_d, "wv": wv_d, "wo": wo_d}
            w_f32 = {}
            for nm in ("wq", "wk", "wv", "wo"):
                w_f32[nm] = consts.tile([128, NC, D], F32, tag=f"{nm}f",
                                        name=f"{nm}f")

            def dma_w(nm):
                nc.gpsimd.dma_start(
                    out=w_f32[nm],
                    in_=w_dram[nm].rearrange("(kc p) e -> p kc e", p=128))
            wq_sb = consts.tile([128, NC, D], BF16, tag="wqb")
            wk_sb = consts.tile([128, NC, D], BF16, tag="wkb")
            wv_sb = consts.tile([128, NC, D], BF16, tag="wvb")
            wo_bf = consts.tile([128, NC, D], BF16, tag="wob")
            bo128 = consts.tile([128, D], F32, tag="bo")
            nc.sync.dma_start(
                out=bo128,
                in_=bass.AP(tensor=bo_d.tensor, offset=bo_d.offset,
                            ap=[[0, 128], [1, D]]))

            # ---- double-buffered per-bank tensors --------------------
            # qbd: block-diagonal q^T: [d(128=2 heads), c, i(8), j(128=2
            # heads)]; off-diagonal 64x64 blocks stay zero forever.
            qbd = [bankp.tile([128, NC, 8, 128], BF16, tag=f"qbd{p}",
                              name=f"qbd{p}") for p in range(2)]
            nc.vector.memzero(qbd[0])
            nc.vector.memzero(qbd[1])
            kTb = [bankp.tile([128, NC, D], BF16, tag=f"kTb{p}",
                              name=f"kTb{p}") for p in range(2)]
            xTb = [bankp.tile([128, NC, D], BF16, tag=f"xTb{p}",
                              name=f"xTb{p}") for p in range(2)]
            vb = [bankp.tile([128, 4, D], BF16, tag=f"vb{p}",
                             name=f"vb{p}") for p in range(2)]
            otb = [bankp.tile([128, NC, D], BF16, tag=f"otb{p}",
                              name=f"otb{p}") for p in range(2)]

            xin_tiles = {}
            pbs = {}
            # ---- stages ----------------------------------------------
            def dma_bank(bk):
                for t in range(4):
                    nt = bk * 4 + t
                    xt = xin.tile([128, D], F32, tag="xt", name=f"xt{nt}")
                    nc.sync.dma_start(
                        out=xt, in_=x_d[nt * 128:(nt + 1) * 128, :])
                    if bk < 3:
                        # banks 0-2 transpose straight from fp32 (startup
                        # critical path: skip the convert stage)
                        xin_tiles[nt] = xt
                        continue
                    xb = xbfp.tile([128, D], BF16, tag="xb",
                                   name=f"xb{nt}")
                    if t % 2 == 0:
                        nc.vector.tensor_copy(out=xb, in_=xt)
                    else:
                        nc.scalar.copy(out=xb, in_=xt)
                    xin_tiles[nt] = xb

            def trans_tile(bk, t):
                nt = bk * 4 + t
                xt = xin_tiles.pop(nt)
                if bk < 3:
                    tpf = psum.tile([128, D], F32, tag="pj", bufs=2,
                                    name=f"tpf{nt}")
                    for kc in range(NC):
                        nc.tensor.transpose(
                            tpf[:, kc * 128:(kc + 1) * 128],
                            xt[:, kc * 128:(kc + 1) * 128],
                            ident_f32)
                    src_ap = bass.AP(tensor=tpf.tensor, offset=tpf.offset,
                                     ap=[[tpf.ap[0][0], 128], [128, 4],
                                         [1, 128]])
                else:
                    tp = psum.tile([128, 4, 128], BF16, tag="tp", bufs=2,
                                   name=f"tp{nt}")
                    for kc in range(NC):
                        nc.tensor.transpose(
                            tp[:, kc, :], xt[:, kc * 128:(kc + 1) * 128],
                            ident_bf)
                    src_ap = tp
                # one strided cast: [128, kc, 128] -> xTb[:, kc, t*128:...]
                dstT = xTb[bk % 2]
                pstride = dstT.ap[0][0]
                dst = bass.AP(tensor=dstT.tensor,
                              offset=dstT.offset + t * 128,
                              ap=[[pstride, 128], [D, 4], [1, 128]])
                nc.vector.tensor_copy(out=dst, in_=src_ap)

            def proj_qk(bk, which):
                wsb = wq_sb if which == "q" else wk_sb
                for c in range(NC):
                    pj = psum.tile([128, D], F32, tag="pj", bufs=2,
                                   name=f"p{which}{bk}_{c}")
                    for kc in range(NC):
                        nc.tensor.matmul(
                            pj,
                            wsb[:, kc, c * 128:(c + 1) * 128],
                            xTb[bk % 2][:, kc, :],
                            start=(kc == 0), stop=(kc == NC - 1))
                    if which == "k":
                        nc.scalar.copy(out=kTb[bk % 2][:, c, :], in_=pj)
                    else:
                        # two strided copies into the block-diagonal qbd
                        dstT = qbd[bk % 2]
                        pstride = dstT.ap[0][0]
                        pjst = pj.ap[0][0]
                        for h in range(2):
                            src = bass.AP(
                                tensor=pj.tensor,
                                offset=pj.offset + h * 64 * pjst,
                                ap=[[pjst, 64], [64, 8], [1, 64]])
                            dst = bass.AP(
                                tensor=dstT.tensor,
                                offset=(dstT.offset + h * 64 * pstride
                                        + c * 8 * 128 + h * 64),
                                ap=[[pstride, 64], [128, 8], [1, 64]])
                            if h == 0:
                                nc.vector.tensor_copy(out=dst, in_=src)
                            else:
                                nc.scalar.copy(out=dst, in_=src)

            def proj_v(bk, t0, t1):
                for t in (t0, t1):
                    pv = psum.tile([128, D], F32, tag="pj", bufs=2,
                                   name=f"pv{bk}_{t}")
                    for kc in range(NC):
                        nc.tensor.matmul(
                            pv,
                            xTb[bk % 2][:, kc, t * 128:(t + 1) * 128],
                            wv_sb[:, kc, :],
                            start=(kc == 0), stop=(kc == NC - 1))
                    nc.scalar.copy(out=vb[bk % 2][:, t, :], in_=pv)

            def front(bk, c):
                """S matmuls (block-diag, full width) + softmax chain."""
                sbt = psum.tile([128, 8, G], F32, tag="sb", bufs=2,
                                name=f"sb{bk}_{c}")
                for s in range(8):
                    nc.tensor.matmul(
                        sbt[:, s, :],
                        qbd[bk % 2][:, c, s, :],
                        kTb[bk % 2][:, c, s * G:(s + 1) * G],
                        start=True, stop=True)
                eb = attn.tile([128, 8, G], BF16, tag="eb", bufs=6,
                               name=f"eb{bk}_{c}")
                nc.scalar.activation(
                    out=eb, in_=sbt,
                    func=mybir.ActivationFunctionType.Exp,
                    scale=SCALE)
                sums = attn.tile([128, 8], F32, tag="sums", bufs=6,
                                 name=f"sums{bk}_{c}")
                nc.vector.reduce_sum(out=sums, in_=eb,
                                     axis=mybir.AxisListType.X)
                rec = attn.tile([128, 8], F32, tag="rec", bufs=6,
                                name=f"rec{bk}_{c}")
                nc.vector.reciprocal(out=rec, in_=sums)
                rec_b = bass.AP(tensor=rec.tensor, offset=rec.offset,
                                ap=[rec.ap[0], rec.ap[1], [0, G]])
                pb = attn.tile([128, 8, G], BF16, tag="pb", bufs=6,
                               name=f"pb{bk}_{c}")
                nc.gpsimd.tensor_mul(pb, eb, rec_b)
                pbs[(bk, c)] = pb

            def mid(bk, c):
                """P transposes -> pts in SBUF (bf16 in, fp32 psum out)."""
                pb = pbs.pop((bk, c))
                ptp = psum.tile([128, 4, 128], BF16, tag="tp", bufs=2,
                                name=f"ptp{bk}_{c}")
                for p2 in range(4):
                    nc.tensor.transpose(
                        ptp[:, p2, :], pb[:, 2 * p2:2 * p2 + 2, :],
                        ident_bf)
                pts = attn.tile([128, 4, 128], BF16, tag="pts",
                                bufs=4, name=f"pts{bk}_{c}")
                nc.scalar.copy(out=pts, in_=ptp)
                pbs[(bk, c, "pts")] = pts

            def back(bk, c):
                """O matmuls, extract diagonal blocks into otb."""
                pts = pbs.pop((bk, c, "pts"))
                obe = [psum.tile([128, 4, 128], F32, tag="ob", bufs=2,
                                 name=f"ob{bk}_{c}_{e}") for e in range(2)]
                for p2 in range(4):
                    for e in range(2):
                        nc.tensor.matmul(
                            obe[e][:, p2, :],
                            vb[bk % 2][e * 64:e * 64 + 64, p2,
                                       c * 128:(c + 1) * 128],
                            pts[e * 64:e * 64 + 64, p2, :],
                            start=True, stop=True,
                            tile_position=(e * 64, 0))
                # extract per-head diagonal blocks: 4 copies (h x e)
                dstT = otb[bk % 2]
                pstride = dstT.ap[0][0]
                for h in range(2):
                    for e in range(2):
                        src_t = obe[e]
                        sstr = src_t.ap[0][0]
                        src = bass.AP(
                            tensor=src_t.tensor,
                            offset=(src_t.offset + h * 64 * sstr
                                    + h * 64),
                            ap=[[sstr, 64], [128, 4], [1, 64]])
                        dst = bass.AP(
                            tensor=dstT.tensor,
                            offset=(dstT.offset + h * 64 * pstride
                                    + c * D + e * 64),
                            ap=[[pstride, 64], [128, 4], [1, 64]])
                        if h == 0:
                            nc.vector.tensor_copy(out=dst, in_=src)
                        else:
                            nc.scalar.copy(out=dst, in_=src)

            def final_nt(bk, t):
                nt = bk * 4 + t
                fp = psum.tile([128, D], F32, tag="pj", bufs=2,
                               name=f"fp{nt}")
                for c in range(NC):
                    nc.tensor.matmul(
                        fp, otb[bk % 2][:, c, t * 128:(t + 1) * 128],
                        wo_bf[:, c, :],
                        start=(c == 0), stop=(c == NC - 1))
                osb = outp.tile([128, D], F32, tag="osb",
                                name=f"osb{nt}")
                nc.vector.tensor_add(osb, fp, bo128)
                nc.sync.dma_start(
                    out=out_d[nt * 128:(nt + 1) * 128, :], in_=osb)

            # ---- schedule --------------------------------------------
            # prologue: banks 0 and 1 in flight before the steady loop
            dma_bank(0)
            dma_w("wq")
            dma_bank(1)
            dma_w("wk")
            dma_w("wv")
            nc.scalar.copy(out=wq_sb, in_=w_f32["wq"])
            for t in range(4):
                trans_tile(0, t)
            nc.scalar.copy(out=wk_sb, in_=w_f32["wk"])
            nc.scalar.copy(out=wv_sb, in_=w_f32["wv"])

            # steady state at step bk: project bank P=bk, attend A=bk-1,
            # write out F=bk-2, transpose L=bk+1, dma bank bk+2.
            for bk in range(NB + 2):
                P = bk if bk < NB else None
                A = bk - 1 if 0 <= bk - 1 < NB else None
                F = bk - 2 if 0 <= bk - 2 < NB else None
                L = bk + 1 if bk + 1 < NB else None
                M = bk + 2 if bk + 2 < NB else None

                if M is not None:
                    dma_bank(M)
                if A is not None:
                    front(A, 0)
                if P is not None:
                    proj_qk(P, "q")
                if A is not None:
                    front(A, 1)
                if P is not None:
                    proj_qk(P, "k")
                if A is not None:
                    mid(A, 0)
                    front(A, 2)
                if P is not None:
                    proj_v(P, 0, 1)
                if A is not None:
                    back(A, 0)
                    mid(A, 1)
                    front(A, 3)
                if P is not None:
                    proj_v(P, 2, 3)
                if A is not None:
                    back(A, 1)
                    mid(A, 2)
                if L is not None:
                    trans_tile(L, 0)
                    trans_tile(L, 1)
                if A is not None:
                    back(A, 2)
                    mid(A, 3)
                if L is not None:
                    trans_tile(L, 2)
                    trans_tile(L, 3)
                if A is not None:
                    back(A, 3)
                if F is not None:
                    for t in range(4):
                        final_nt(F, t)
                if bk == 0:
                    dma_w("wo")
                if bk == 1:
                    nc.scalar.mul(out=wo_bf, in_=w_f32["wo"], mul=2.0)

    nc.compile()
    return nc


_CACHED = None


def _get_nc():
    global _CACHED
    if _CACHED is None:
        _CACHED = build_kernel()
    return _CACHED


def run(inputs: dict, trace: bool = False):
    from concourse.bass_utils import run_bass_kernel_spmd
    nc = _get_nc()
    x = np.ascontiguousarray(inputs["x"], dtype=np.float32)
    b = x.shape[0]
    shared = {k: np.ascontiguousarray(inputs[k], dtype=np.float32)
              for k in ("Wq", "Wk", "Wv", "Wo", "bo")}
    in_maps = [{"x": x[i], **shared} for i in range(b)]
    res = run_bass_kernel_spmd(nc, in_maps, list(range(b)), trace=trace)
    out = np.stack([res.results[i]["out"] for i in range(b)], axis=0)
    return out, res


def kernel(**inputs) -> np.ndarray:
    out, _ = run(inputs, trace=False)
    return out.astype(np.float32)
